# revision 14
# baseline (speedup 1.0000x reference)
"""GraphSAGE 2-layer forward on 8 TRN2 NeuronCores — scatter-add variant.

Strategy (graph/data parallel per sharding hint):
- Nodes dst-sharded across 8 cores (6250 nodes/core, 49 tiles of 128).
- x is sharded host-side: each core receives only its own 6250-row bf16
  shard; the full bf16 feature table is assembled ON DEVICE via AllGather.
- Host sorts edges by dst and splits them by src<32768 (dma_gather idx is
  int16), then slices the stream into ROUNDS: round r holds the r-th edge
  of each dst node, so dst indices are unique within a round. Per round:
  gpsimd.dma_gather pulls x[src]/p[src] rows into SBUF, DVE upcasts to
  f32, and gpsimd.dma_scatter_add segment-sums them into a zeroed DRAM
  accumulator indexed by local dst (pad slots scatter into a trash tile).
  Rounds are serialized by barriers because dma_scatter_add does not
  serialize same-row read-modify-write (verified on HW: duplicate indices
  lose updates). This replaces the one-hot-matmul scatter of the earlier
  revision and shrinks the program ~2x (walrus recompiles the NEFF on
  every call under the axon redirect, so BIR size is wall-clock).
- x ships as int8 (device dequant, scale baked into the NEFF) and the
  dense weights ship sharded 8 ways and are AllGathered on device; with
  the bf16 output this cuts host->device traffic ~15x vs the baseline.
- L1 per dst tile: mean = agg * 1/deg; DMA-transpose mean and the own-x
  tile; dense W1_l/W1_r matmuls (bf16) with fused bias+relu into h1T.
- h kept transposed [hid, nodes] bf16 in SBUF; p = h @ W2_l computed
  row-major, AllGathered (bf16, 128-col padded rows) so every core can
  gather p[src].
- L2: same gather/scatter machinery on p; + h @ W2_r + b2; log_softmax
  along the free dim; bf16 out, upcast on host.
"""

import numpy as np
import ml_dtypes

import concourse.bacc as bacc
import concourse.bass as bass
import concourse.mybir as mybir
import concourse.tile as tile
from concourse.bass_utils import run_bass_kernel_spmd

N = 50000
F = 128
HID = 256
CLS = 47
CORES = 8
NPC = N // CORES           # 6250
TPC = (NPC + 127) // 128   # 49 tiles per core
SPLIT = 32768              # int16 index limit for dma_gather
TRASH = TPC * 128          # first trash row of the dram accumulator
AGGR = TPC * 128 + 128     # accumulator rows incl. trash tile

f32 = mybir.dt.float32
bf16 = mybir.dt.bfloat16
i16 = mybir.dt.int16
i8 = mybir.dt.int8
ALU = mybir.AluOpType
ACTF = mybir.ActivationFunctionType

# weight blob layout (bf16): w1l | w1r | w2l_pad | w2r_pad. Each region
# is padded to a 128-multiple column width so SBUF loads map affinely.
# Only core-invariant data may live here: the blob is reassembled on
# device from DIFFERENT cores' shards by the AllGather.
_W1N = F * HID                      # 32768 = 256 blob rows
_W2N = 128 * 128                    # w2l padded [128, 94->128]
_WROWS = (2 * _W1N + 2 * _W2N) // 128            # 768 rows, /8 = 96
_WSH = _WROWS // 8                  # rows per core


def _rup128(v):
    return (int(v) + 127) // 128 * 128


def _host_prep(x, edge_index):
    src = np.asarray(edge_index[0], np.int64)
    dst = np.asarray(edge_index[1], np.int64)
    deg = np.bincount(dst, minlength=N).astype(np.float32)

    order = np.argsort(dst, kind="stable")
    src_s = src[order]
    dst_s = dst[order]
    bounds = np.searchsorted(dst_s, np.arange(0, N + 1, NPC))

    # dma_scatter_add does NOT serialize same-row RMW: duplicate dst
    # indices within one scatter lose updates (verified on HW). Split the
    # edge stream into rounds — round r holds the r-th edge of each dst —
    # so indices are unique per scatter; rounds are serialized by barriers.
    per_core = []
    for c in range(CORES):
        sl = slice(bounds[c], bounds[c + 1])
        sc = src_s[sl]
        dl = dst_s[sl] - c * NPC
        rank = np.arange(len(dl)) - np.searchsorted(dl, dl)
        seg = (sc >= SPLIT).astype(np.int64)
        o2 = np.lexsort((dl, seg, rank))
        per_core.append((sc[o2], dl[o2], seg[o2], rank[o2]))

    R = 1 + max(int(pc[3].max()) for pc in per_core)
    cnt = np.zeros((CORES, R, 2), np.int64)
    for c in range(CORES):
        np.add.at(cnt[c], (per_core[c][3], per_core[c][2]), 1)
    Ns = ((cnt.max(axis=0) + 127) // 128) * 128          # [R, 2]

    rounds = []
    off = 0
    for r in range(R):
        rounds.append((off, int(Ns[r, 0]), int(Ns[r, 1])))
        off += int(Ns[r, 0] + Ns[r, 1])
    total = off
    cols = total // 16

    gidx_all, sidx_all, degp_all, xsh_all = [], [], [], []
    xscale = max(float(np.abs(x).max()), 1e-30) / 127.0
    xq = np.clip(np.round(x / xscale), -127, 127).astype(np.int8)
    trash = TRASH + (np.arange(total) % 128)
    for c in range(CORES):
        gstream = np.zeros(total, np.int64)   # pad gathers row 0
        sstream = trash.copy()                # pad scatters -> trash
        sc, dl, seg, rank = per_core[c]
        # per-(round, seg) contiguous slices of the lexsorted edge arrays
        csum = np.concatenate([[0], np.cumsum(cnt[c].reshape(-1))])
        for r in range(R):
            a = rounds[r][0]
            for s in (0, 1):
                i0, i1 = csum[r * 2 + s], csum[r * 2 + s + 1]
                n = i1 - i0
                if n == 0:
                    continue
                p0 = a if s == 0 else a + int(Ns[r, 0])
                gv = sc[i0:i1] - (SPLIT if s == 1 else 0)
                gstream[p0:p0 + n] = gv
                sstream[p0:p0 + n] = dl[i0:i1]
        gidx_all.append(np.ascontiguousarray(
            gstream.reshape(-1, 16).T.astype(np.int16)))
        sidx_all.append(np.ascontiguousarray(
            sstream.reshape(-1, 16).T.astype(np.int16)))
        dpc = np.ones(TPC * 128, np.float32)
        dpc[:NPC] = deg[c * NPC:(c + 1) * NPC]
        degp_all.append(np.ascontiguousarray(dpc.reshape(TPC, 128).T))
        xp = np.zeros((TPC * 128, F), np.int8)
        xp[:NPC] = xq[c * NPC:(c + 1) * NPC]
        xsh_all.append(xp)

    maxc = max((nl + nh) // 128 for (_, nl, nh) in rounds)
    sched = dict(cols=cols, rounds=rounds, maxc=maxc, xscale=xscale)
    return sched, gidx_all, sidx_all, degp_all, xsh_all


def _build(sched):
    rounds, cols, maxc = sched["rounds"], sched["cols"], sched["maxc"]
    xscale = sched["xscale"]

    nc = bacc.Bacc("TRN2", num_devices=CORES)
    xsh_h = nc.declare_dram_parameter("xsh", [TPC * 128, F], i8, False)
    gidx_h = nc.declare_dram_parameter("gidx", [16, cols], i16, False)
    sidx_h = nc.declare_dram_parameter("sidx", [16, cols], i16, False)
    wsh_h = nc.declare_dram_parameter("wsh", [_WSH, 128], bf16, False)
    degp_h = nc.declare_dram_parameter("degp", [128, TPC], bf16, False)
    b1_h = nc.declare_dram_parameter("b1c", [128, 2], f32, False)
    b2_h = nc.declare_dram_parameter("b2r", [1, CLS], f32, False)
    out_h = nc.declare_dram_parameter("out", [NPC, CLS], bf16, True)

    x_loc = nc.dram_tensor("x_loc", [TPC * 128, F], bf16)
    x_full = nc.dram_tensor("x_full", [N, F], bf16, addr_space="Shared")
    w_loc = nc.dram_tensor("w_loc", [_WSH, 128], bf16)
    w_full = nc.dram_tensor("w_full", [_WROWS, 128], bf16,
                            addr_space="Shared")
    p_loc = nc.dram_tensor("p_loc", [NPC, 128], bf16)
    p_full = nc.dram_tensor("p_full", [N, 128], bf16, addr_space="Shared")
    agg1 = nc.dram_tensor("agg1", [AGGR, F], f32)
    agg2 = nc.dram_tensor("agg2", [AGGR, 128], f32)

    with tile.TileContext(nc) as tc:
        with (
            tc.tile_pool(name="const", bufs=1) as cp,
            tc.tile_pool(name="msg", bufs=2) as msgp,
            tc.tile_pool(name="msgf", bufs=2) as msgfp,
            tc.tile_pool(name="sb", bufs=3) as sbp,
            tc.tile_pool(name="small", bufs=4) as smp,
        ):
            # ---- dequantize the int8 x shard to bf16, then AllGather ----
            # (collectives cannot read IO tensors: stage via internal dram)
            for r0 in range(0, TPC * 128, 1024):
                rr = min(1024, TPC * 128 - r0)
                a = rr // 128
                xi = sbp.tile([128, 1024], i8, tag="xq")
                nc.sync.dma_start(
                    xi[:, 0:rr].rearrange("p (a f) -> p a f", f=F),
                    xsh_h[r0:r0 + rr, :].rearrange("(a b) f -> b a f",
                                                   b=128))
                xb = sbp.tile([128, 1024], bf16, tag="xb")
                nc.scalar.activation(xb[:, 0:rr], xi[:, 0:rr], ACTF.Copy,
                                     bias=0.0, scale=float(xscale))
                nc.sync.dma_start(
                    x_loc[r0:r0 + rr, :].rearrange("(a b) f -> b a f",
                                                   b=128),
                    xb[:, 0:rr].rearrange("p (a f) -> p a f", f=F))
            nc.gpsimd.collective_compute(
                "AllGather", ALU.bypass,
                replica_groups=[list(range(CORES))],
                ins=[x_loc[0:NPC, :].opt()], outs=[x_full.ap().opt()])
            # ---- weights travel sharded too: AllGather the blob ----
            nc.sync.dma_start(w_loc.ap(), wsh_h[:, :])
            nc.gpsimd.collective_compute(
                "AllGather", ALU.bypass,
                replica_groups=[list(range(CORES))],
                ins=[w_loc.ap().opt()], outs=[w_full.ap().opt()])

            # ---- persistent tiles ----
            # dma_gather/scatter want the idx block replicated across the
            # 8 Q7 cores (16 partitions each) — replicate on device.
            gidx_sb = cp.tile([128, cols], i16, tag="gidx")
            sidx_sb = cp.tile([128, cols], i16, tag="sidx")
            for idx_sb, idx_h in ((gidx_sb, gidx_h), (sidx_sb, sidx_h)):
                nc.sync.dma_start(idx_sb[0:16, :], idx_h[:, :])
                nc.sync.dma_start(idx_sb[16:32, :], idx_sb[0:16, :])
                nc.sync.dma_start(idx_sb[32:64, :], idx_sb[0:32, :])
                nc.sync.dma_start(idx_sb[64:128, :], idx_sb[0:64, :])

            b1_sb = cp.tile([128, 2], f32, tag="b1")
            nc.sync.dma_start(b1_sb[:], b1_h[:, :])
            b2_sb = cp.tile([1, CLS], f32, tag="b2")
            nc.sync.dma_start(b2_sb[:], b2_h[:, :])

            # weight loads read the AllGathered blob — barrier first
            tc.strict_bb_all_engine_barrier()
            w1l_sb = cp.tile([F, HID], bf16, tag="w1l")
            w1r_sb = cp.tile([F, HID], bf16, tag="w1r")
            w2l_sb = cp.tile([128, 128], bf16, tag="w2l")
            w2r_sb = cp.tile([128, 128], bf16, tag="w2r")
            o = 0
            for wt, nel in ((w1l_sb, _W1N), (w1r_sb, _W1N),
                            (w2l_sb, _W2N), (w2r_sb, _W2N)):
                rows = nel // 128
                a = rows // 128
                if a > 1:
                    nc.sync.dma_start(
                        wt[:].rearrange("p (a f) -> p a f", f=128),
                        w_full[o:o + rows, :].rearrange("(p a) f -> p a f",
                                                        a=a))
                else:
                    nc.sync.dma_start(wt[:], w_full[o:o + rows, :])
                o += rows
            degb_sb = cp.tile([128, TPC], bf16, tag="degb")
            nc.sync.dma_start(degb_sb[:], degp_h[:, :])
            deg_sb = cp.tile([128, TPC], f32, tag="deg")
            nc.vector.tensor_copy(deg_sb[:], degb_sb[:])

            invc_sb = cp.tile([128, TPC], f32, tag="invc")
            nc.vector.tensor_scalar(invc_sb[:], deg_sb[:], 1.0, None, ALU.max)
            inv_sb = cp.tile([128, TPC], f32, tag="inv")
            nc.vector.reciprocal_approx_fast(inv_sb[:], invc_sb[:])

            ones_sb = cp.tile([1, 128], f32, tag="ones")
            nc.vector.memset(ones_sb[:], 1.0)

            # ---- zero the dram accumulators (written once per layer) ----
            zsb = cp.tile([128, 1024], f32, tag="zero")
            nc.vector.memset(zsb[:], 0.0)
            for agg in (agg1, agg2):
                for r0 in range(0, AGGR, 1024):
                    rr = min(1024, AGGR - r0)
                    nc.sync.dma_start(
                        agg[r0:r0 + rr, :].rearrange("r f -> f r"),
                        zsb[:, 0:rr])

            h1T0 = cp.tile([128, TPC * 128], bf16, tag="h1a")
            h1T1 = cp.tile([128, TPC * 128], bf16, tag="h1b")

            # zero-fills and the x AllGather staging must land before any
            # scatter-add / gather touches the dram tensors.
            tc.strict_bb_all_engine_barrier()

            def sweep(table_full, agg_dram):
                """gather rows -> upcast f32 -> scatter-add into agg_dram.

                One scatter per round (unique dst indices); a barrier
                before each scatter serializes the same-row RMW between
                rounds while letting the next round's gather overlap."""
                for (a, n_lo, n_hi) in rounds:
                    C = (n_lo + n_hi) // 128
                    msg = msgp.tile([128, maxc * F], bf16, tag="msg")
                    msg3 = msg[:].rearrange("p (c e) -> p c e", e=F)
                    if n_lo:
                        nc.gpsimd.dma_gather(
                            msg3[:, 0:n_lo // 128, :], table_full[0:SPLIT, :],
                            gidx_sb[:, a // 16:(a + n_lo) // 16],
                            n_lo, n_lo, F, single_packet=False)
                    if n_hi:
                        nc.gpsimd.dma_gather(
                            msg3[:, n_lo // 128:C, :], table_full[SPLIT:N, :],
                            gidx_sb[:, (a + n_lo) // 16:(a + n_lo + n_hi) // 16],
                            n_hi, n_hi, F, single_packet=False)
                    msgf = msgfp.tile([128, maxc * F], f32, tag="msgf")
                    nc.vector.tensor_copy(msgf[:, 0:C * F], msg[:, 0:C * F])
                    msgf3 = msgf[:].rearrange("p (c e) -> p c e", e=F)
                    tc.strict_bb_all_engine_barrier()
                    nc.gpsimd.dma_scatter_add(
                        agg_dram[:, :], msgf3[:, 0:C, :],
                        sidx_sb[:, a // 16:(a + n_lo + n_hi) // 16],
                        n_lo + n_hi, n_lo + n_hi, F, single_packet=False)

            # =============== Layer 1 ===============
            # scatter-add writes to dram are not visible to the tile
            # dependency tracker — hard barrier before the agg reads.
            sweep(x_full, agg1)
            tc.strict_bb_all_engine_barrier()
            # 4 dst tiles per iteration: one 3D dma + one broadcast scale,
            # per-half z accumulates into a full-bank [128, 512] PSUM tile
            # (rhs free dim 512); tile 48 handled singly at the end.
            BL = 4
            with tc.tile_pool(name="zp", bufs=2, space="PSUM") as zpp:
                for tb in range(0, TPC - 1, BL):
                    agg_sb = sbp.tile([128, BL * F], f32, tag="agg")
                    nc.sync.dma_start(
                        agg_sb[:].rearrange("p (b f) -> p b f", f=F),
                        agg1[tb * 128:(tb + BL) * 128, :].rearrange(
                            "(b p) f -> p b f", p=128))
                    mean = sbp.tile([128, BL * F], bf16, tag="mean")
                    nc.vector.tensor_tensor(
                        mean[:].rearrange("p (b f) -> p b f", f=F),
                        agg_sb[:].rearrange("p (b f) -> p b f", f=F),
                        inv_sb[:, tb:tb + BL].rearrange(
                            "p b -> p b ()").to_broadcast((128, BL, F)),
                        ALU.mult)
                    meanT = sbp.tile([128, BL * 128], bf16, tag="meanT")
                    xoT = sbp.tile([128, BL * 128], bf16, tag="xoT")
                    for k in range(BL):
                        ks = slice(k * 128, (k + 1) * 128)
                        nc.sync.dma_start_transpose(meanT[:, ks], mean[:, ks])
                        nc.sync.dma_start_transpose(
                            xoT[:, ks],
                            x_loc[(tb + k) * 128:(tb + k + 1) * 128, :])
                    for h, h1T in ((0, h1T0), (1, h1T1)):
                        z_ps = zpp.tile([128, BL * 128], f32, tag="z")
                        nc.tensor.matmul(z_ps[:],
                                         w1l_sb[:, h * 128:(h + 1) * 128],
                                         meanT[:], start=True, stop=False)
                        nc.tensor.matmul(z_ps[:],
                                         w1r_sb[:, h * 128:(h + 1) * 128],
                                         xoT[:], start=False, stop=True)
                        nc.scalar.activation(
                            h1T[:, tb * 128:(tb + BL) * 128], z_ps[:],
                            ACTF.Relu, bias=b1_sb[:, h:h + 1], scale=1.0)
                for t in (TPC - 1,):
                    agg_sb = sbp.tile([128, F], f32, tag="agg1l")
                    nc.sync.dma_start(agg_sb[:], agg1[t * 128:(t + 1) * 128, :])
                    mean = sbp.tile([128, F], bf16, tag="mean1l")
                    nc.vector.tensor_scalar(mean[:], agg_sb[:],
                                            inv_sb[:, t:t + 1], None, ALU.mult)
                    meanT = sbp.tile([128, 128], bf16, tag="meanT1l")
                    nc.sync.dma_start_transpose(meanT[:], mean[:])
                    xoT = sbp.tile([128, 128], bf16, tag="xoT1l")
                    nc.sync.dma_start_transpose(
                        xoT[:], x_loc[t * 128:(t + 1) * 128, :])
                    z_ps = zpp.tile([128, 256], f32, tag="z1l")
                    for h, h1T in ((0, h1T0), (1, h1T1)):
                        zs = z_ps[:, h * 128:(h + 1) * 128]
                        nc.tensor.matmul(zs, w1l_sb[:, h * 128:(h + 1) * 128],
                                         meanT[:], start=True, stop=False)
                        nc.tensor.matmul(zs, w1r_sb[:, h * 128:(h + 1) * 128],
                                         xoT[:], start=False, stop=True)
                        nc.scalar.activation(h1T[:, t * 128:(t + 1) * 128],
                                             zs, ACTF.Relu,
                                             bias=b1_sb[:, h:h + 1],
                                             scale=1.0)

            # =============== p = h @ W2_l, AllGather ===============
            with tc.tile_pool(name="pp", bufs=2, space="PSUM") as ppp:
                for t in range(TPC):
                    ts = slice(t * 128, (t + 1) * 128)
                    pp_ps = ppp.tile([128, 64], f32, tag="pp")
                    nc.tensor.matmul(pp_ps[:, 0:CLS], h1T0[:, ts],
                                     w2l_sb[:, 0:CLS], start=True, stop=False)
                    nc.tensor.matmul(pp_ps[:, 0:CLS], h1T1[:, ts],
                                     w2l_sb[:, CLS:2 * CLS], start=False,
                                     stop=True)
                    psb = sbp.tile([128, 128], bf16, tag="psb")
                    nc.vector.memset(psb[:, CLS:128], 0.0)
                    nc.scalar.activation(psb[:, 0:CLS], pp_ps[:, 0:CLS],
                                         ACTF.Copy)
                    rows = NPC - t * 128 if t == TPC - 1 else 128
                    nc.sync.dma_start(p_loc[t * 128:t * 128 + rows, :],
                                      psb[0:rows, :])

                nc.gpsimd.collective_compute(
                    "AllGather", ALU.bypass,
                    replica_groups=[list(range(CORES))],
                    ins=[p_loc.ap().opt()], outs=[p_full.ap().opt()])

                # b2 broadcast across partitions via rank-1 matmul
                b2_ps = ppp.tile([128, 64], f32, tag="pp")
                nc.tensor.matmul(b2_ps[:, 0:CLS], ones_sb[0:1, :],
                                 b2_sb[0:1, :], start=True, stop=True)
                b2bc = cp.tile([128, CLS], f32, tag="b2bc")
                nc.scalar.activation(b2bc[:], b2_ps[:, 0:CLS], ACTF.Copy)

            # =============== Layer 2 ===============
            sweep(p_full, agg2)
            tc.strict_bb_all_engine_barrier()
            # 49 dst tiles processed 7 at a time: matmuls accumulate into
            # one wide PSUM tile; the softmax chain runs on [128, 7*CLS]
            # blocks with per-block scalars via stride-0 broadcast APs.
            BT = 7

            def b3(ap):
                return ap.rearrange("p (b c) -> p b c", c=CLS)

            def sc3(ap):
                return ap.rearrange("p b -> p b ()").to_broadcast(
                    (128, BT, CLS))

            with tc.tile_pool(name="op", bufs=2, space="PSUM") as opp:
                for tb in range(0, TPC, BT):
                    o_ps = opp.tile([128, BT * CLS], f32, tag="op")
                    for k in range(BT):
                        t = tb + k
                        tsl = slice(t * 128, (t + 1) * 128)
                        os = o_ps[:, k * CLS:(k + 1) * CLS]
                        nc.tensor.matmul(os, h1T0[:, tsl], w2r_sb[:, 0:CLS],
                                         start=True, stop=False)
                        nc.tensor.matmul(os, h1T1[:, tsl],
                                         w2r_sb[:, CLS:2 * CLS],
                                         start=False, stop=True)
                    agg_sb = smp.tile([128, BT * CLS], f32, tag="agg2")
                    nc.sync.dma_start(
                        b3(agg_sb[:]),
                        agg2[tb * 128:(tb + BT) * 128, 0:CLS].rearrange(
                            "(b p) c -> p b c", p=128))
                    s_sb = smp.tile([128, BT * CLS], f32, tag="s")
                    nc.vector.tensor_tensor(
                        b3(s_sb[:]), b3(agg_sb[:]),
                        inv_sb[:, tb:tb + BT].rearrange(
                            "p b -> p b ()").to_broadcast((128, BT, CLS)),
                        ALU.mult)
                    lg = smp.tile([128, BT * CLS], f32, tag="lg")
                    nc.vector.tensor_tensor(lg[:], o_ps[:], s_sb[:], ALU.add)
                    lg2 = smp.tile([128, BT * CLS], f32, tag="lg2")
                    nc.vector.tensor_tensor(
                        b3(lg2[:]), b3(lg[:]),
                        b2bc[:].rearrange("p c -> p () c").to_broadcast(
                            (128, BT, CLS)), ALU.add)
                    mx = smp.tile([128, BT], f32, tag="mx")
                    nc.vector.tensor_reduce(
                        mx[:].rearrange("p b -> p b ()"), b3(lg2[:]),
                        mybir.AxisListType.X, ALU.max)
                    sh = smp.tile([128, BT * CLS], f32, tag="sh")
                    nc.vector.tensor_tensor(b3(sh[:]), b3(lg2[:]),
                                            sc3(mx[:]), ALU.subtract)
                    ex = smp.tile([128, BT * CLS], f32, tag="ex")
                    nc.scalar.activation(ex[:], sh[:], ACTF.Exp)
                    sm = smp.tile([128, BT], f32, tag="sm")
                    nc.vector.tensor_reduce(
                        sm[:].rearrange("p b -> p b ()"), b3(ex[:]),
                        mybir.AxisListType.X, ALU.add)
                    ls = smp.tile([128, BT], f32, tag="ls")
                    nc.scalar.activation(ls[:], sm[:], ACTF.Ln)
                    res = smp.tile([128, BT * CLS], bf16, tag="res")
                    nc.vector.tensor_tensor(b3(res[:]), b3(sh[:]),
                                            sc3(ls[:]), ALU.subtract)
                    # full 128-row tiles in one 3D DMA; tile 48's 106-row
                    # tail separately
                    nfull = BT if tb + BT < TPC else BT - 1
                    nc.sync.dma_start(
                        out_h[tb * 128:(tb + nfull) * 128, :].rearrange(
                            "(b p) c -> p b c", p=128),
                        b3(res[:])[:, 0:nfull, :])
                    if nfull < BT:
                        rows = NPC - (tb + nfull) * 128
                        nc.sync.dma_start(
                            out_h[(tb + nfull) * 128:NPC, :],
                            res[0:rows, nfull * CLS:(nfull + 1) * CLS])

    nc.compile()
    return nc


def _make_in_maps(inputs, gidx_all, sidx_all, degp_all, xsh_all):
    w1l = np.asarray(np.asarray(inputs["W1_l"], np.float32),
                     ml_dtypes.bfloat16)
    w1r = np.asarray(np.asarray(inputs["W1_r"], np.float32),
                     ml_dtypes.bfloat16)
    w2lf = np.asarray(inputs["W2_l"], np.float32)
    w2rf = np.asarray(inputs["W2_r"], np.float32)
    w2l = np.ascontiguousarray(
        np.concatenate([w2lf[:128, :], w2lf[128:, :]], axis=1)).astype(
            ml_dtypes.bfloat16)
    w2r = np.ascontiguousarray(
        np.concatenate([w2rf[:128, :], w2rf[128:, :]], axis=1)).astype(
            ml_dtypes.bfloat16)
    w2lp = np.zeros((128, 128), ml_dtypes.bfloat16)
    w2lp[:, 0:2 * CLS] = w2l
    w2rp = np.zeros((128, 128), ml_dtypes.bfloat16)
    w2rp[:, 0:2 * CLS] = w2r
    blob = np.zeros(_WROWS * 128, ml_dtypes.bfloat16)
    o = 0
    for a in (w1l, w1r, w2lp, w2rp):
        blob[o:o + a.size] = a.reshape(-1)
        o += a.size
    blob2 = blob.reshape(_WROWS, 128)
    b1c = np.ascontiguousarray(
        np.asarray(inputs["b1"], np.float32).reshape(2, 128).T)
    b2r = np.ascontiguousarray(
        np.asarray(inputs["b2"], np.float32).reshape(1, CLS))
    in_maps = []
    for c in range(CORES):
        in_maps.append({
            "xsh": xsh_all[c],
            "gidx": gidx_all[c],
            "sidx": sidx_all[c],
            "wsh": np.ascontiguousarray(blob2[c * _WSH:(c + 1) * _WSH]),
            "degp": degp_all[c].astype(ml_dtypes.bfloat16),
            "b1c": b1c, "b2r": b2r,
        })
    return in_maps


def _run(inputs, trace=False):
    x = np.asarray(inputs["x"], np.float32)
    edge_index = np.asarray(inputs["edge_index"])
    sched, gidx_all, sidx_all, degp_all, xsh_all = _host_prep(x, edge_index)
    nc = _build(sched)
    in_maps = _make_in_maps(inputs, gidx_all, sidx_all, degp_all, xsh_all)
    res = run_bass_kernel_spmd(nc, in_maps, core_ids=list(range(CORES)),
                               trace=trace)
    out = np.concatenate(
        [np.asarray(r["out"], np.float32) for r in res.results], axis=0)
    return out, res


def kernel(**inputs):
    out, _ = _run(inputs, trace=False)
    return out
